# revision 1
# baseline (speedup 1.0000x reference)
"""Pre-norm multi-head self-attention layer (LN -> QKV -> softmax(QK^T)V -> Wo + residual)
for Trainium2, SPMD over 8 NeuronCores.

Sharding: core = (batch b, q-chunk c) with b in 0..3, c in 0..1.
Each core receives the full batch-b token set (for K/V) plus its 1024-token
q-chunk, computes attention for all 8 heads over its chunk, and writes the
[1024, 512] output slice. Host concatenates slices — no collectives.

Layout strategy (per core):
  - LN in [tok, d], PE-transpose to xT [d, tok] (fp32 transpose, exact-ish)
  - qT/kT [hidden, tok] via W.T @ xT; v [tok, hidden] via xT.T @ W
  - scores transposed: sT[j, i] = kT_h.T-slice @ qT_h (K=64, two heads run on
    64-row PE tiles concurrently)
  - exp on ScalarE from PSUM [128, 1024] tiles (scale folded into activation)
  - attnV: ones-augmented v ([j, 65] stationary) accumulates both the head
    output and the softmax row-sums in one PSUM [65, 1024] tensor
  - normalization: row-sum -> DRAM -> partition-broadcast DMA -> reciprocal
    -> one multiply on the evacuated head output
  - out-projection per 64-row head chunk (K=64), + bias + residual, store.
"""

import numpy as np

import concourse.bacc as bacc
import concourse.bass as bass
import concourse.tile as tile
from concourse import mybir
from concourse.bass_utils import run_bass_kernel_spmd
from concourse.masks import make_identity

f32 = mybir.dt.float32
f32r = mybir.dt.float32r
AF = mybir.ActivationFunctionType
ALU = mybir.AluOpType
ts = bass.ts

B, N, D = 4, 2048, 512
H, DH = 8, 64
HP = 4                 # head pairs
CHUNK = N // 2         # q rows per core
KT = D // 128          # contraction tiles over d / hidden
NT_B = N // 128        # xb token tiles
NT_Q = CHUNK // 128
TB = N // 512          # 512-token blocks for kT/v moving dim
QB = CHUNK // 512
SM_SCALE = float((H * DH) ** -0.5)
EPS = 1e-5

_CACHED = {}


def _build(has_bqk, has_bv, has_bo):
    nc = bacc.Bacc("TRN2", target_bir_lowering=False)

    xb = nc.dram_tensor("xb", [N, D], f32, kind="ExternalInput")
    xq = nc.dram_tensor("xq", [CHUNK, D], f32, kind="ExternalInput")
    wq = nc.dram_tensor("wq", [D, D], f32r, kind="ExternalInput")
    wk = nc.dram_tensor("wk", [D, D], f32r, kind="ExternalInput")
    wv = nc.dram_tensor("wv", [D, D], f32r, kind="ExternalInput")
    wo = nc.dram_tensor("wo", [D, D], f32r, kind="ExternalInput")
    bqk = nc.dram_tensor("bqk", [2, D], f32, kind="ExternalInput")   # bias_q, bias_k rows
    bvo = nc.dram_tensor("bvo", [2, D], f32, kind="ExternalInput")   # bias_v, bo rows
    out = nc.dram_tensor("out", [CHUNK, D], f32, kind="ExternalOutput")
    scr = nc.dram_tensor("scr", [H, CHUNK], f32r)

    with tile.TileContext(nc) as tc:
        with tc.tile_pool(name="consts", bufs=1) as consts, \
             tc.tile_pool(name="persist", bufs=1) as persist:

            ident = consts.tile([128, 128], f32)
            make_identity(nc, ident)
            eps_sb = consts.tile([128, 1], f32)
            nc.vector.memset(eps_sb, EPS)

            wq_sb = consts.tile([128, KT, D], f32r)
            wk_sb = consts.tile([128, KT, D], f32r)
            wv_sb = consts.tile([128, KT, D], f32r)
            wo_sb = consts.tile([64, H, D], f32r)
            if has_bqk:
                bq_sb = consts.tile([128, KT], f32)   # bias_q column per kt
                bk_sb = consts.tile([128, KT], f32)
                nc.sync.dma_start(out=bq_sb, in_=bqk[0, :].rearrange("(k p) -> p k", p=128))
                nc.sync.dma_start(out=bk_sb, in_=bqk[1, :].rearrange("(k p) -> p k", p=128))
            if has_bv:
                bv_sb = consts.tile([128, D], f32)
                nc.gpsimd.dma_start(
                    out=bv_sb,
                    in_=bass.AP(tensor=bvo.tensor if hasattr(bvo, "tensor") else bvo,
                                offset=0, ap=[[0, 128], [1, D]]))
            if has_bo:
                bo_sb = consts.tile([128, D], f32)
                nc.gpsimd.dma_start(
                    out=bo_sb,
                    in_=bass.AP(tensor=bvo.tensor if hasattr(bvo, "tensor") else bvo,
                                offset=D, ap=[[0, 128], [1, D]]))

            kT_sb = persist.tile([128, HP, N], f32r)
            qT_sb = persist.tile([128, HP, CHUNK], f32r)
            v_sb = persist.tile([128, NT_B, H, DH + 1], f32r)
            ones_st = consts.tile([128, NT_B * H], f32)
            nc.vector.memset(ones_st, 1.0)
            nc.vector.tensor_copy(
                out=v_sb[:, :, :, DH:DH + 1],
                in_=ones_st.rearrange("p (t h) -> p t h", t=NT_B)[:, :, :, None])

            def ln_tile(lnp, psP, src_ap, dst, dst_col):
                xt = lnp.tile([128, D], f32, tag="xt")
                nc.sync.dma_start(out=xt, in_=src_ap)
                st = lnp.tile([128, 6], f32, tag="st")
                nc.vector.bn_stats(out=st, in_=xt)
                mv = lnp.tile([128, 2], f32, tag="mv")
                nc.vector.bn_aggr(out=mv, in_=st)
                rstd = lnp.tile([128, 1], f32, tag="rstd")
                nc.scalar.activation(out=rstd, in_=mv[:, 1:2], func=AF.Sqrt,
                                     bias=eps_sb, scale=1.0)
                nc.vector.reciprocal(out=rstd, in_=rstd)
                xn = lnp.tile([128, D], f32, tag="xn")
                nc.vector.tensor_scalar(out=xn, in0=xt, scalar1=mv[:, 0:1],
                                        scalar2=rstd, op0=ALU.subtract, op1=ALU.mult)
                ps = psP.tile([128, D], f32, tag="a")
                for kt in range(KT):
                    nc.tensor.transpose(ps[:, ts(kt, 128)], xn[:, ts(kt, 128)], ident)
                nc.scalar.copy(
                    out=dst[:, :, ts(dst_col, 128)],
                    in_=ps.rearrange("p (k t) -> p k t", k=KT))

            # ---------------- xq LN + qT (own scope; frees xqT early) ------
            with tc.tile_pool(name="lnq", bufs=8) as lnq, \
                 tc.tile_pool(name="xtq", bufs=1) as xtq, \
                 tc.tile_pool(name="psQ", bufs=3, space="PSUM") as psQ:
                xqT = xtq.tile([128, KT, CHUNK], f32r)
                for i in range(NT_Q):
                    ln_tile(lnq, psQ, xq[ts(i, 128), :], xqT, i)
                for kt in range(KT):
                    nc.sync.dma_start(out=wq_sb[:, kt, :], in_=wq[ts(kt, 128), :])
                for hp in range(HP):
                    for qb in range(QB):
                        ps = psQ.tile([128, 512], f32, tag="a")
                        for kt in range(KT):
                            nc.tensor.matmul(ps, lhsT=wq_sb[:, kt, ts(hp, 128)],
                                             rhs=xqT[:, kt, ts(qb, 512)],
                                             start=(kt == 0), stop=(kt == KT - 1))
                        if has_bqk:
                            nc.vector.tensor_scalar(
                                out=qT_sb[:, hp, ts(qb, 512)], in0=ps,
                                scalar1=bq_sb[:, hp:hp + 1], scalar2=None, op0=ALU.add)
                        else:
                            nc.scalar.copy(out=qT_sb[:, hp, ts(qb, 512)], in_=ps)

            # ------- xb LN + v, then per-head-pair kT + attention ----------
            with tc.tile_pool(name="lnb", bufs=8) as lnb, \
                 tc.tile_pool(name="xtb", bufs=1) as xtb, \
                 tc.tile_pool(name="psP", bufs=4, space="PSUM") as psP:
                xbT = xtb.tile([128, KT, N], f32r)
                # stream in groups of 4 token tiles: LN -> v -> kT so PE can
                # enter attention as soon as the first tb-block is complete
                for g in range(TB):
                    for i in range(4 * g, 4 * g + 4):
                        ln_tile(lnb, psP, xb[ts(i, 128), :], xbT, i)
                    if g == 0:
                        for kt in range(KT):
                            nc.sync.dma_start(out=wv_sb[:, kt, :], in_=wv[ts(kt, 128), :])
                            nc.sync.dma_start(out=wk_sb[:, kt, :], in_=wk[ts(kt, 128), :])
                    for tcn in range(4 * g, 4 * g + 4):
                        ps = psP.tile([128, 512], f32, tag="a")
                        for kt in range(KT):
                            nc.tensor.matmul(ps, lhsT=xbT[:, kt, ts(tcn, 128)],
                                             rhs=wv_sb[:, kt, :],
                                             start=(kt == 0), stop=(kt == KT - 1))
                        if has_bv:
                            nc.vector.tensor_add(out=ps, in0=ps, in1=bv_sb)
                        nc.scalar.copy(
                            out=v_sb[:, tcn, :, 0:DH],
                            in_=ps.rearrange("p (h d) -> p h d", h=H))
                    for hp in range(HP):
                        ps = psP.tile([128, 512], f32, tag="a")
                        for kt in range(KT):
                            nc.tensor.matmul(ps, lhsT=wk_sb[:, kt, ts(hp, 128)],
                                             rhs=xbT[:, kt, ts(g, 512)],
                                             start=(kt == 0), stop=(kt == KT - 1))
                        if has_bqk:
                            nc.vector.tensor_scalar(
                                out=kT_sb[:, hp, ts(g, 512)], in0=ps,
                                scalar1=bk_sb[:, hp:hp + 1], scalar2=None,
                                op0=ALU.add)
                        else:
                            nc.scalar.copy(out=kT_sb[:, hp, ts(g, 512)],
                                           in_=ps)

            # ---------------- attention ----------------
            for h in range(H):
                nc.sync.dma_start(out=wo_sb[:, h, :], in_=wo[ts(h, 64), :])
            otp = tc.alloc_tile_pool(name="ot", bufs=1)
            oT = [otp.tile([DH + 1, CHUNK], f32r, name=f"oT{h}", tag=f"o{h}")
                  for h in range(H)]
            with tc.tile_pool(name="pt", bufs=6) as ptp, \
                 tc.tile_pool(name="rt", bufs=2) as rtp, \
                 tc.tile_pool(name="psS", bufs=3, space="PSUM") as psS, \
                 tc.tile_pool(name="psO", bufs=1, space="PSUM") as psO:

                for hp in range(HP):
                    for hs in range(2):
                            h = 2 * hp + hs
                            lo, hi = 64 * hs, 64 * hs + 64
                            po = psO.tile([DH + 1, CHUNK], f32, tag="po")
                            for jt in range(NT_B):
                                sT = psS.tile([128, CHUNK], f32, tag="sT")
                                for qb in range(QB):
                                    nc.tensor.matmul(sT[:, ts(qb, 512)],
                                                     lhsT=kT_sb[lo:hi, hp, ts(jt, 128)],
                                                     rhs=qT_sb[lo:hi, hp, ts(qb, 512)],
                                                     start=True, stop=True)
                                pT = ptp.tile([128, CHUNK], f32r, tag="pT")
                                nc.scalar.activation(out=pT, in_=sT, func=AF.Exp,
                                                     scale=SM_SCALE)
                                for qb in range(QB):
                                    nc.tensor.matmul(po[:, ts(qb, 512)],
                                                     lhsT=v_sb[:, jt, h, :],
                                                     rhs=pT[:, ts(qb, 512)],
                                                     start=(jt == 0),
                                                     stop=(jt == NT_B - 1))
                            # unnormalized evac + rowsum -> DRAM -> bcast -> recip
                            nc.vector.tensor_copy(out=oT[h], in_=po)
                            nc.sync.dma_start(out=scr[h, :], in_=oT[h][DH:DH + 1, :])
                            rT = rtp.tile([64, CHUNK], f32r, tag="rT")
                            nc.gpsimd.dma_start(
                                out=rT,
                                in_=bass.AP(tensor=scr.tensor if hasattr(scr, "tensor") else scr,
                                            offset=h * CHUNK, ap=[[0, 64], [1, CHUNK]]))
                            with nc.allow_low_precision(reason="f32r is bit-identical to f32"):
                                nc.vector.reciprocal(out=rT, in_=rT)
                            nc.vector.tensor_mul(out=oT[h][0:DH, :],
                                                 in0=oT[h][0:DH, :], in1=rT)

            # ---------------- out projection + residual ----------------
            # all 8 PSUM banks are free here: accumulate heads 0-6 for every
            # i-tile first so the PE works through them while head 7's
            # normalize chain (DRAM round-trip) completes, then append h7.
            with tc.tile_pool(name="fin", bufs=1) as fin, \
                 tc.tile_pool(name="psF", bufs=1, space="PSUM") as psF:
                ps_list = []
                res_list = []
                for it in range(NT_Q):
                    res = fin.tile([128, D], f32, name=f"res{it}", tag=f"res{it}")
                    nc.sync.dma_start(out=res, in_=xq[ts(it, 128), :])
                    res_list.append(res)
                for it in range(NT_Q):
                    ps = psF.tile([128, D], f32, name=f"f{it}", tag=f"f{it}")
                    for h in range(H - 1):
                        nc.tensor.matmul(ps, lhsT=oT[h][0:DH, ts(it, 128)],
                                         rhs=wo_sb[:, h, :],
                                         start=(h == 0), stop=False)
                    ps_list.append(ps)
                for it in range(NT_Q):
                    nc.tensor.matmul(ps_list[it], lhsT=oT[H - 1][0:DH, ts(it, 128)],
                                     rhs=wo_sb[:, H - 1, :],
                                     start=False, stop=True)
                    fo = fin.tile([128, D], f32, name=f"fo{it}", tag="fosb", bufs=4)
                    nc.vector.tensor_add(out=fo, in0=ps_list[it], in1=res_list[it])
                    if has_bo:
                        nc.vector.tensor_add(out=fo, in0=fo, in1=bo_sb)
                    nc.sync.dma_start(out=out[ts(it, 128), :], in_=fo)
            otp.release()

    nc.finalize()
    return nc


def kernel(x, ln_g, ln_b, Wq, Wk, Wv, Wo, bo):
    x = np.ascontiguousarray(np.asarray(x, dtype=np.float32))
    ln_g = np.asarray(ln_g, dtype=np.float32)
    ln_b = np.asarray(ln_b, dtype=np.float32)
    Wq = np.asarray(Wq, dtype=np.float32)
    Wk = np.asarray(Wk, dtype=np.float32)
    Wv = np.asarray(Wv, dtype=np.float32)
    Wo = np.asarray(Wo, dtype=np.float32)
    bo = np.asarray(bo, dtype=np.float32)

    # fold LN affine into the projection weights (exact algebra):
    #   xn = xhat * g + b  =>  xn @ W = xhat @ (g[:,None]*W) + (b @ W)
    Wq_f = (ln_g[:, None] * Wq).astype(np.float32)
    Wk_f = (ln_g[:, None] * Wk).astype(np.float32)
    Wv_f = (ln_g[:, None] * Wv).astype(np.float32)
    bq = (ln_b.astype(np.float64) @ Wq.astype(np.float64)).astype(np.float32)
    bk = (ln_b.astype(np.float64) @ Wk.astype(np.float64)).astype(np.float32)
    bv = (ln_b.astype(np.float64) @ Wv.astype(np.float64)).astype(np.float32)

    has_bqk = bool(np.any(bq) or np.any(bk))
    has_bv = bool(np.any(bv))
    has_bo = bool(np.any(bo))

    key = (has_bqk, has_bv, has_bo)
    if key not in _CACHED:
        _CACHED[key] = _build(*key)
    nc = _CACHED[key]

    bqk = np.stack([bq, bk]).astype(np.float32)
    bvo = np.stack([bv, bo]).astype(np.float32)

    in_maps = []
    for core in range(8):
        b, c = core // 2, core % 2
        in_maps.append({
            "xb": np.ascontiguousarray(x[b]),
            "xq": np.ascontiguousarray(x[b, c * CHUNK:(c + 1) * CHUNK]),
            "wq": Wq_f, "wk": Wk_f, "wv": Wv_f, "wo": Wo,
            "bqk": bqk, "bvo": bvo,
        })

    res = run_bass_kernel_spmd(nc, in_maps, core_ids=list(range(8)))
    global LAST_RESULT
    LAST_RESULT = res
    full = np.empty((B, N, D), dtype=np.float32)
    for core in range(8):
        b, c = core // 2, core % 2
        full[b, c * CHUNK:(c + 1) * CHUNK] = res.results[core]["out"]
    return full

